# revision 1
# baseline (speedup 1.0000x reference)
"""BilateralRotation Trainium2 kernel: out[b,c] = R1[c] @ wkv[b,c] @ R2[c],
R = Cayley(p) = (I - A)(I + A)^-1, A = 0.5(p - p^T).

Sharding: 8 NeuronCores, head-parallel — core k owns heads [4k, 4k+4) for all
512 batches (32 MB in / 32 MB out per core; the tiny per-head rotations are
computed on-device per core via Newton-Schulz).

Device program per core:
  Phase 1 — Cayley on-device: B = A^T A (PE), M = I + B (SPD),
    Newton-Schulz X' = 2X - X(M X) with X0 = I/300 -> X = M^-1 (15 iters,
    fp32; X^T tracked via exact PE transposes so skew error cancels),
    R = C X with C = I - 2A - B.
  Phase 2 — bilateral rotation, fully pipelined:
    - input DMA in a folded layout (each partition holds two consecutive
      h-rows = 512B contiguous elements; 128 partitions span the core's 4
      heads with a single affine stride) -> full DMA bandwidth
    - MM1 (Y = R1 X): two parity-split accumulating matmuls per head-pair,
      stationary = blockdiag of parity-sliced R1^T, moving = data, float32r
    - T1: PE 128x128 transposes (4 items each)
    - MM2 (Z^T = R2^T Y^T): stationary blockdiag(R2,R2), strided rhs
      gathering one head's columns, N=512, float32r
    - Z^T tiles dumped contiguously to HBM; the host inverts the (fixed,
      known) index permutation while unsharding/concatenating the 8 shards.
"""

import sys
import types
from contextlib import ExitStack

import numpy as np

# ---------------------------------------------------------------------------
# TileContext patch: this walrus build accepts only ONE sync-wait per
# instruction; hoist extra waits onto nops inserted before the instruction.
# ---------------------------------------------------------------------------
import concourse.bass as bass
import concourse.tile as tile
from concourse.vector_clock import ScopedClock
from concourse import masks, mybir
from concourse.bass_utils import run_bass_kernel_spmd

WAIT_LIMIT = 1


def _hoist_extra_waits(nc, inst, hint):
    nops = []
    si = inst.sync_info
    if si is not None and len(si.on_wait) > WAIT_LIMIT:
        extras = si.on_wait[:-WAIT_LIMIT]
        del si.on_wait[:-WAIT_LIMIT]
        for w in extras:
            nop = nc.engines[inst.engine].nop(nofuse=True, hint=hint)
            nsi = nop.ins.sync_info
            if nsi is None:
                nop.ins.sync_info = mybir.SyncInfo(on_wait=[w], on_update=[])
            else:
                nsi.on_wait.append(w)
            nops.append(nop.ins)
    return nops


def _split_waits(nc):
    cur_list = nc.cur_bb.bb.instructions
    for f in nc.m.functions:
        for bb in f.blocks:
            orig = list(bb.instructions)
            if not any(i.sync_info and len(i.sync_info.on_wait) > WAIT_LIMIT
                       for i in orig):
                continue
            new_list = []
            for inst in orig:
                nops = _hoist_extra_waits(nc, inst, "split_wait")
                for nop in nops:
                    if cur_list and cur_list[-1] is nop:
                        cur_list.pop()
                    else:
                        cur_list.remove(nop)
                new_list.extend(nops)
                new_list.append(inst)
            bb.instructions[:] = new_list


def _drain_and_barrier(self, tick_clock, wait_clock):
    nc = self.nc
    _split_waits(nc)
    drain_inst = nc.sync.drain()
    wait_clock.add_sem_waits(drain_inst.ins,
                             ScopedClock({None: tick_clock.global_clock}))
    nops = _hoist_extra_waits(nc, drain_inst.ins, "drain_split_wait")
    if nops:
        insts = nc.cur_bb.bb.instructions
        di = insts.index(drain_inst.ins)
        insts.append(insts.pop(di))
    nc.all_engine_barrier()
    assert self.sems is not None
    popped = nc._tile_sem_poison_stack.pop()
    assert popped is self._sem_poison
    nc.clear_and_free_semaphores(list(self.sems.allocated().values()))
    nc.all_engine_barrier()


tile.TileContext._drain_and_barrier = _drain_and_barrier

# ---------------------------------------------------------------------------
# Program builder
# ---------------------------------------------------------------------------
dt = mybir.dt
F32 = dt.float32
F32R = dt.float32r

HPC = 4                     # heads per core
B = 512
H = W = 64
BSTRIDE = HPC * H * W
CSTRIDE = H * W
NG = 32                     # batch groups of 16
N_CORES = 8


def build(mm_f32r=True, ns_iters=15, c0=1.0 / 300.0,
          in_bufs=8, out_bufs=3, mid_bufs=2):
    nc = bass.Bass("TRN2", target_bir_lowering=False, debug=False,
                   num_devices=N_CORES)
    mmdt = F32R if mm_f32r else F32
    wkv = nc.dram_tensor("wkv", [B, HPC, H, W], mmdt, kind="ExternalInput")
    p_left = nc.dram_tensor("p_left", [HPC, H, H], F32, kind="ExternalInput")
    p_right = nc.dram_tensor("p_right", [HPC, W, W], F32,
                             kind="ExternalInput")
    out = nc.dram_tensor("out_scr", [NG, 128, 2048], F32,
                         kind="ExternalOutput")

    with tile.TileContext(nc) as tc, ExitStack() as ctx:
        const_pool = ctx.enter_context(tc.tile_pool(name="const", bufs=1))
        bd_pool = ctx.enter_context(tc.tile_pool(name="bd", bufs=1))

        ident = const_pool.tile([128, 128], F32, tag="ident")
        masks.make_identity(nc, ident[:])
        i64 = ident[0:64, 0:64]
        zeros = const_pool.tile([128, 128], F32, tag="zeros")
        nc.gpsimd.memset(zeros[:], 0.0)

        bdl = {}
        for P in range(2):
            for s in range(2):
                t = bd_pool.tile([128, 128], mmdt, tag=f"bdl{P}{s}")
                nc.vector.tensor_copy(t[:], zeros[:])
                bdl[(P, s)] = t
        bdr = []
        for c in range(HPC):
            t = bd_pool.tile([128, 128], mmdt, tag=f"bdr{c}")
            nc.vector.tensor_copy(t[:], zeros[:])
            bdr.append(t)

        # ---------------- Phase 1: Newton-Schulz Cayley ----------------
        with ExitStack() as nsctx:
            ns_sb = nsctx.enter_context(tc.tile_pool(name="ns_sb", bufs=2))
            ns_keep = nsctx.enter_context(tc.tile_pool(name="ns_keep",
                                                       bufs=2))
            ns_ps = nsctx.enter_context(
                tc.tile_pool(name="ns_ps", bufs=1, space="PSUM"))

            xs, xts, cts, ms = [], [], [], []
            for m in range(2 * HPC):
                side, c = divmod(m, HPC)
                src = p_left if side == 0 else p_right

                psb = ns_sb.tile([64, 64], F32, tag="p_in")
                nc.sync.dma_start(psb[:], src.ap()[c])

                ptp = ns_ps.tile([64, 64], F32, tag=f"nsp{m}")
                nc.tensor.transpose(ptp[:], psb[:], i64)

                asb = ns_keep.tile([64, 64], F32, tag=f"a{m}")
                nc.vector.tensor_sub(asb[:], psb[:], ptp[:])
                nc.vector.tensor_scalar_mul(asb[:], asb[:], 0.5)   # A

                bps = ns_ps.tile([64, 64], F32, tag=f"nsp{m}")
                nc.tensor.matmul(bps[:], asb[:], asb[:])           # B = A^T A
                msb = ns_keep.tile([64, 64], F32, tag=f"m{m}")
                nc.vector.tensor_add(msb[:], bps[:], i64)          # M = I + B

                ctsb = ns_keep.tile([64, 64], F32, tag=f"ct{m}")
                nc.vector.scalar_tensor_tensor(                    # 2A - B
                    ctsb[:], asb[:], 2.0, bps[:],
                    op0=mybir.AluOpType.mult, op1=mybir.AluOpType.subtract)
                nc.vector.tensor_add(ctsb[:], ctsb[:], i64)        # C^T

                xsb = ns_keep.tile([64, 64], F32, tag=f"x{m}")
                nc.vector.tensor_scalar_mul(xsb[:], i64, c0)       # X0
                xtsb = ns_keep.tile([64, 64], F32, tag=f"xt{m}")
                nc.vector.tensor_scalar_mul(xtsb[:], i64, c0)

                xs.append(xsb)
                xts.append(xtsb)
                cts.append(ctsb)
                ms.append(msb)

            for k in range(ns_iters):
                for m in range(2 * HPC):
                    ups = ns_ps.tile([64, 64], F32, tag=f"nsp{m}")
                    nc.tensor.matmul(ups[:], ms[m][:], xs[m][:])   # U = M X
                    usb = ns_sb.tile([64, 64], F32, tag=f"ns_u{m}")
                    if m % 2 == 0:
                        nc.vector.tensor_copy(usb[:], ups[:])
                    else:
                        nc.scalar.copy(usb[:], ups[:])
                    wps = ns_ps.tile([64, 64], F32, tag=f"nsp{m}")
                    nc.tensor.matmul(wps[:], xts[m][:], usb[:])    # W = X U
                    xnew = ns_keep.tile([64, 64], F32, tag=f"x{m}")
                    nc.vector.scalar_tensor_tensor(                # 2X - W
                        xnew[:], xs[m][:], 2.0, wps[:],
                        op0=mybir.AluOpType.mult,
                        op1=mybir.AluOpType.subtract)
                    xs[m] = xnew
                    xtps = ns_ps.tile([64, 64], F32, tag=f"nsp{m}")
                    nc.tensor.transpose(xtps[:], xnew[:], i64)
                    xtnew = ns_keep.tile([64, 64], F32, tag=f"xt{m}")
                    if m % 2 == 0:
                        nc.scalar.copy(xtnew[:], xtps[:])
                    else:
                        nc.vector.tensor_copy(xtnew[:], xtps[:])
                    xts[m] = xtnew

            for c in range(HPC):
                r1ps = ns_ps.tile([64, 64], F32, tag=f"nsp{c}")
                nc.tensor.matmul(r1ps[:], cts[c][:], xs[c][:])     # R1 = C X
                r1sb = ns_sb.tile([64, 64], F32, tag=f"r1_{c}")
                nc.vector.tensor_copy(r1sb[:], r1ps[:])
                P, hh = divmod(c, 2)
                for s in range(2):
                    # (R1[:, s::2])^T = parity-s rows of R1^T  -> [32, 64]
                    sl = bass.AP(r1sb.tensor, r1sb.offset + s,
                                 [list(r1sb.ap[0]), [2, 32]])
                    tps = ns_ps.tile([32, 64], F32, tag=f"nsp{c}")
                    nc.tensor.transpose(tps[:], sl, i64)
                    dst = bdl[(P, s)]
                    nc.vector.tensor_copy(
                        dst[64 * P + 32 * hh:64 * P + 32 * hh + 32,
                            64 * hh:64 * hh + 64], tps[:])

                mr = HPC + c
                r2ps = ns_ps.tile([64, 64], F32, tag=f"nsp{mr}")
                nc.tensor.matmul(r2ps[:], cts[mr][:], xs[mr][:])   # R2 = C X
                nc.vector.tensor_copy(bdr[c][0:64, 0:64], r2ps[:])
                nc.vector.tensor_copy(bdr[c][64:128, 64:128], r2ps[:])

        # ---------------- Phase 2: main loop ----------------
        io_pool = ctx.enter_context(tc.tile_pool(name="io", bufs=in_bufs))
        out_pool = ctx.enter_context(tc.tile_pool(name="outp", bufs=out_bufs))
        mid_pool = ctx.enter_context(tc.tile_pool(name="mid", bufs=mid_bufs))
        ps_pool = ctx.enter_context(
            tc.tile_pool(name="mainps", bufs=1, space="PSUM"))

        for g in range(NG):
            xin = io_pool.tile([128, 2048], mmdt, tag="xin")
            nc.sync.dma_start(
                xin[:], bass.AP(wkv, 16 * g * BSTRIDE,
                                [[128, 128], [BSTRIDE, 16], [1, 128]]))

            ysb = [mid_pool.tile([128, 1024], F32, tag=f"ysb{P}",
                                 name=f"ysb{P}_{g}") for P in range(2)]
            for half in range(2):
                for P in range(2):
                    yps = ps_pool.tile([128, 512], F32, tag=f"mm1_{P}",
                                       bufs=2)
                    for s in range(2):
                        base = xin[64 * P:64 * P + 64,
                                   1024 * half + 64 * s:
                                   1024 * half + 64 * s + 64]
                        rhs = bass.AP(base.tensor, base.offset,
                                      [list(base.ap[0]), [128, 8], [1, 64]])
                        nc.tensor.matmul(
                            yps[:], bdl[(P, s)][64 * P:64 * P + 64, :], rhs,
                            start=(s == 0), stop=(s == 1),
                            tile_position=(64 * P, 0))
                    dstv = ysb[P][:, 512 * half:512 * half + 512]
                    if (half + P) % 2 == 0:
                        nc.vector.tensor_copy(dstv, yps[:])
                    else:
                        nc.scalar.copy(dstv, yps[:])

            ytsb = [mid_pool.tile([128, 1024], mmdt, tag=f"ytsb{P}",
                                  name=f"ytsb{P}_{g}") for P in range(2)]
            for P in range(2):
                for hp in range(2):
                    tps = ps_pool.tile([128, 512], F32, tag="t1", bufs=2)
                    for q in range(4):
                        qq = 4 * hp + q
                        nc.tensor.transpose(
                            tps[:, 128 * q:128 * q + 128],
                            ysb[P][:, 128 * qq:128 * qq + 128], ident[:])
                    dstv = ytsb[P][:, 512 * hp:512 * hp + 512]
                    if (P + hp) % 2 == 0:
                        nc.vector.tensor_copy(dstv, tps[:])
                    else:
                        nc.scalar.copy(dstv, tps[:])

            zsb = out_pool.tile([128, 2048], F32, tag="zsb")
            for c in range(HPC):
                P, hh = divmod(c, 2)
                zps = ps_pool.tile([128, 512], F32, tag="mm2", bufs=2)
                base = ytsb[P][:, 64 * hh:64 * hh + 64]
                rhs = bass.AP(base.tensor, base.offset,
                              [list(base.ap[0]), [128, 8], [1, 64]])
                nc.tensor.matmul(zps[:], bdr[c][:], rhs)
                dstv = zsb[:, 512 * c:512 * c + 512]
                if c % 2 == 0:
                    nc.vector.tensor_copy(dstv, zps[:])
                else:
                    nc.scalar.copy(dstv, zps[:])

            nc.sync.dma_start(
                bass.AP(out, g * 128 * 2048, [[2048, 128], [1, 2048]]),
                zsb[:])

    return nc


def _unscramble(scr):
    """scr [NG, 128, 2048] -> [512, 4, 64, 64].
    scr[g, 64*bp + j, 512*h + 64*q + i] = Z[16g + 2q + bp, h][i, j]."""
    a = scr.reshape(NG, 2, 64, HPC, 8, 64)      # g, bp, j, h, q, i
    a = a.transpose(0, 4, 1, 3, 5, 2)           # g, q, bp, h, i, j
    return np.ascontiguousarray(a.reshape(B, HPC, H, W))


_CACHED = {}


def _get_program():
    if "nc" not in _CACHED:
        _CACHED["nc"] = build()
    return _CACHED["nc"]


def kernel(wkv, p_left, p_right):
    wkv = np.ascontiguousarray(wkv, dtype=np.float32)
    p_left = np.ascontiguousarray(p_left, dtype=np.float32)
    p_right = np.ascontiguousarray(p_right, dtype=np.float32)
    assert wkv.shape == (B, 32, H, W), wkv.shape

    nc = _get_program()
    in_maps = []
    for k in range(N_CORES):
        sl = slice(HPC * k, HPC * k + HPC)
        in_maps.append({
            "wkv": np.ascontiguousarray(wkv[:, sl]),
            "p_left": np.ascontiguousarray(p_left[sl]),
            "p_right": np.ascontiguousarray(p_right[sl]),
        })
    res = run_bass_kernel_spmd(nc, in_maps, list(range(N_CORES)))
    return np.concatenate(
        [_unscramble(np.asarray(res.results[k]["out_scr"]))
         for k in range(N_CORES)], axis=1)



# revision 8
# speedup vs baseline: 1.7716x; 1.7716x over previous
"""BilateralRotation Trainium2 kernel: out[b,c] = R1[c] @ wkv[b,c] @ R2[c],
R = Cayley(p) = (I - A)(I + A)^-1, A = 0.5(p - p^T).

Sharding: 8 NeuronCores, head-parallel - core k owns heads [4k, 4k+4) for all
512 batches. bf16 end-to-end on the data path (rel-err budget 2e-2; measured
~4.5e-3): the host converts the wkv shard to bf16 in a [c, i, b, j] layout so
every DMA line is a 2KB contiguous run, and the device writes bf16 results
that the host casts back to fp32.

Device program per core:
  Phase 1 - Cayley via Newton-Schulz in fp32, 4 pair-packed lanes
    (two 64x64 matrices stacked on 128 partitions; block-diagonal stationaries
    so each NS step is ONE matmul per product):
      B = A^T A, M = I + B, X' = 2X - X(MX), 12 iters, X0 = I/300.
      X is symmetric (polynomial in M), so no X^T tracking is needed.
      R1 path stores R^T = X C^T, R2 path stores R = C X (C^T = I + 2A - B).
  Phase 2 - main loop over 32 groups of 16 batches, all-bf16:
    - MM1: K=128 block-diag stationary blockdiag(R1_c0^T, R1_c1^T) per head
      pair; moving = xin [128, 512] -> Y pair-stacked in PSUM (1 col/cycle,
      full PE).
    - T1: PE transposes of bf16 [128,128] blocks (1 cyc/row vs 2 for fp32).
    - MM2: stationary blockdiag(R2_c, R2_c) (batch-pair K packing), moving
      gathers head c' columns via a strided AP, N=512.
    - PSUM->SBUF copies split across DVE / Act / Pool engines.
    - zsb [128, 2048] bf16 dumped contiguously; host inverts the fixed index
      permutation while unsharding.
"""

import sys
import types
from contextlib import ExitStack

import numpy as np
import ml_dtypes

# ---------------------------------------------------------------------------
# TileContext patch: this walrus build accepts only ONE sync-wait per
# instruction; hoist extra waits onto nops inserted before the instruction.
# ---------------------------------------------------------------------------
import concourse.bass as bass
import concourse.tile as tile
from concourse.vector_clock import ScopedClock
from concourse import masks, mybir
from concourse.bass_utils import run_bass_kernel_spmd

WAIT_LIMIT = 1


def _hoist_extra_waits(nc, inst, hint):
    nops = []
    si = inst.sync_info
    if si is not None and len(si.on_wait) > WAIT_LIMIT:
        extras = si.on_wait[:-WAIT_LIMIT]
        del si.on_wait[:-WAIT_LIMIT]
        for w in extras:
            nop = nc.engines[inst.engine].nop(nofuse=True, hint=hint)
            nsi = nop.ins.sync_info
            if nsi is None:
                nop.ins.sync_info = mybir.SyncInfo(on_wait=[w], on_update=[])
            else:
                nsi.on_wait.append(w)
            nops.append(nop.ins)
    return nops


def _split_waits(nc):
    cur_list = nc.cur_bb.bb.instructions
    for f in nc.m.functions:
        for bb in f.blocks:
            orig = list(bb.instructions)
            if not any(i.sync_info and len(i.sync_info.on_wait) > WAIT_LIMIT
                       for i in orig):
                continue
            new_list = []
            for inst in orig:
                nops = _hoist_extra_waits(nc, inst, "split_wait")
                for nop in nops:
                    if cur_list and cur_list[-1] is nop:
                        cur_list.pop()
                    else:
                        cur_list.remove(nop)
                new_list.extend(nops)
                new_list.append(inst)
            bb.instructions[:] = new_list


def _drain_and_barrier(self, tick_clock, wait_clock):
    nc = self.nc
    _split_waits(nc)
    drain_inst = nc.sync.drain()
    wait_clock.add_sem_waits(drain_inst.ins,
                             ScopedClock({None: tick_clock.global_clock}))
    nops = _hoist_extra_waits(nc, drain_inst.ins, "drain_split_wait")
    if nops:
        insts = nc.cur_bb.bb.instructions
        di = insts.index(drain_inst.ins)
        insts.append(insts.pop(di))
    nc.all_engine_barrier()
    assert self.sems is not None
    popped = nc._tile_sem_poison_stack.pop()
    assert popped is self._sem_poison
    nc.clear_and_free_semaphores(list(self.sems.allocated().values()))
    nc.all_engine_barrier()


tile.TileContext._drain_and_barrier = _drain_and_barrier

# ---------------------------------------------------------------------------
# Program builder
# ---------------------------------------------------------------------------
dt = mybir.dt
F32 = dt.float32
BF16 = dt.bfloat16
BF_NP = np.dtype(ml_dtypes.bfloat16)

HPC = 4                     # heads per core
B = 512
H = W = 64
NG = 32                     # batch groups of 16
GB = 16                     # batches per group
N_CORES = 8
NS_ITERS = 12
C0 = 1.0 / 300.0



def _ecopy(eng, dst, src):
    if hasattr(eng, "tensor_copy"):
        eng.tensor_copy(dst, src)
    else:
        eng.copy(dst, src)


def build(in_bufs=12, out_bufs=3, mid_bufs=2):
    nc = bass.Bass("TRN2", target_bir_lowering=False, debug=False,
                   num_devices=N_CORES)
    # [c, i, b, j] bf16 layout: partition lines are 2KB contiguous runs.
    wkv = nc.dram_tensor("wkv", [HPC, H, B, W], BF16, kind="ExternalInput")
    p_left = nc.dram_tensor("p_left", [HPC, H, H], F32, kind="ExternalInput")
    p_right = nc.dram_tensor("p_right", [HPC, W, W], F32,
                             kind="ExternalInput")
    out = nc.dram_tensor("out_scr", [NG, 128, 2048], BF16,
                         kind="ExternalOutput")

    with tile.TileContext(nc) as tc, ExitStack() as ctx:
        const_pool = ctx.enter_context(tc.tile_pool(name="const", bufs=1))
        bd_pool = ctx.enter_context(tc.tile_pool(name="bd", bufs=1))

        ident = const_pool.tile([128, 128], F32, tag="ident")
        masks.make_identity(nc, ident[:])
        i64 = ident[0:64, 0:64]
        identb = const_pool.tile([128, 128], BF16, tag="identb")
        nc.vector.tensor_copy(identb[:], ident[:])
        zeros = const_pool.tile([128, 128], F32, tag="zeros")
        nc.gpsimd.memset(zeros[:], 0.0)
        # [I64; I64] stacked (both 64x64 blocks of the first 64 cols)
        istk = const_pool.tile([128, 64], F32, tag="istk")
        nc.vector.tensor_copy(istk[0:64, :], i64)
        nc.scalar.copy(istk[64:128, :], i64)

        # main-loop stationaries (bf16): bdl[P] = blockdiag(R1_{2P}^T,
        # R1_{2P+1}^T); bdr[c] = blockdiag(R2_c, R2_c)
        bdl = []
        for P in range(2):
            t = bd_pool.tile([128, 128], BF16, tag=f"bdl{P}")
            nc.vector.tensor_copy(t[:], zeros[:])
            bdl.append(t)
        bdr = []
        for c in range(HPC):
            t = bd_pool.tile([128, 128], BF16, tag=f"bdr{c}")
            nc.vector.tensor_copy(t[:], zeros[:])
            bdr.append(t)

        # ---------------- Phase 1: Newton-Schulz Cayley (pair lanes) -------
        # lane 0,1: p_left pairs (R1); lane 2,3: p_right pairs (R2)
        # (Activation engine has no tensor-tensor ops; arithmetic alternates
        # DVE / Pool per lane, pure copies go to Activation.)
        ENGS = [nc.vector, nc.gpsimd]

        with ExitStack() as nsctx:
            ns_sb = nsctx.enter_context(tc.tile_pool(name="ns_sb", bufs=2))
            ns_keep = nsctx.enter_context(tc.tile_pool(name="ns_keep",
                                                       bufs=2))
            ns_ps = nsctx.enter_context(
                tc.tile_pool(name="ns_ps", bufs=1, space="PSUM"))

            lanes = []
            for L in range(4):
                src = p_left if L < 2 else p_right
                ca, cb = 2 * (L % 2), 2 * (L % 2) + 1

                # side-by-side [64, 128] for one transpose; stacked [128, 64]
                pside = ns_sb.tile([64, 128], F32, tag=f"pside{L}")
                nc.sync.dma_start(pside[:, 0:64], src.ap()[ca])
                nc.sync.dma_start(pside[:, 64:128], src.ap()[cb])
                pstk = ns_sb.tile([128, 64], F32, tag=f"pstk{L}")
                nc.sync.dma_start(pstk[0:64, :], src.ap()[ca])
                nc.sync.dma_start(pstk[64:128, :], src.ap()[cb])

                ptp = ns_ps.tile([128, 64], F32, tag=f"nsps{L}", bufs=2)
                nc.tensor.transpose(ptp[:], pside[:], i64)

                astk = ns_keep.tile([128, 64], F32, tag=f"astk{L}")
                nc.vector.tensor_sub(astk[:], pstk[:], ptp[:])
                nc.vector.tensor_scalar_mul(astk[:], astk[:], 0.5)

                abd = ns_keep.tile([128, 128], F32, tag=f"abd{L}")
                nc.gpsimd.tensor_copy(abd[:], zeros[:])
                nc.gpsimd.tensor_copy(abd[0:64, 0:64], astk[0:64, :])
                nc.gpsimd.tensor_copy(abd[64:128, 64:128], astk[64:128, :])

                bps = ns_ps.tile([128, 64], F32, tag=f"nsps{L}", bufs=2)
                nc.tensor.matmul(bps[:], abd[:], astk[:])      # [A^T A] pair

                mbd = ns_keep.tile([128, 128], F32, tag=f"mbd{L}")
                nc.gpsimd.tensor_copy(mbd[:], zeros[:])
                mstk = ns_sb.tile([128, 64], F32, tag=f"mstk{L}")
                nc.vector.tensor_add(mstk[:], bps[:], istk[:])  # M = I + B
                nc.gpsimd.tensor_copy(mbd[0:64, 0:64], mstk[0:64, :])
                nc.gpsimd.tensor_copy(mbd[64:128, 64:128], mstk[64:128, :])

                ctstk = ns_keep.tile([128, 64], F32, tag=f"ct{L}")
                nc.vector.scalar_tensor_tensor(                # 2A - B
                    ctstk[:], astk[:], 2.0, bps[:],
                    op0=mybir.AluOpType.mult,
                    op1=mybir.AluOpType.subtract)
                nc.vector.tensor_add(ctstk[:], ctstk[:], istk[:])   # C^T

                xstk = ns_keep.tile([128, 64], F32, tag=f"x{L}")
                nc.vector.tensor_scalar_mul(xstk[:], istk[:], C0)
                xbd = ns_keep.tile([128, 128], F32, tag=f"xbd{L}")
                nc.gpsimd.tensor_scalar_mul(xbd[:], ident[:], C0)

                lanes.append(dict(xstk=xstk, xbd=xbd, mbd=mbd,
                                  ctstk=ctstk))

            for k in range(NS_ITERS):
                for L, ln in enumerate(lanes):
                    pps = ns_ps.tile([128, 64], F32, tag=f"nsps{L}", bufs=2)
                    nc.tensor.matmul(pps[:], ln["mbd"][:], ln["xstk"][:])
                    psb = ns_sb.tile([128, 64], F32, tag=f"psb{L}")
                    nc.scalar.copy(psb[:], pps[:])
                    wps = ns_ps.tile([128, 64], F32, tag=f"nsps{L}", bufs=2)
                    nc.tensor.matmul(wps[:], ln["xbd"][:], psb[:])
                    xnew = ns_keep.tile([128, 64], F32, tag=f"x{L}")
                    nc.vector.scalar_tensor_tensor(            # 2X - X(MX)
                        xnew[:], ln["xstk"][:], 2.0, wps[:],
                        op0=mybir.AluOpType.mult,
                        op1=mybir.AluOpType.subtract)
                    ln["xstk"] = xnew
                    if k < NS_ITERS - 1:
                        nc.gpsimd.tensor_copy(ln["xbd"][0:64, 0:64],
                                              xnew[0:64, :])
                        nc.gpsimd.tensor_copy(ln["xbd"][64:128, 64:128],
                                              xnew[64:128, :])

            for L, ln in enumerate(lanes):
                nc.gpsimd.tensor_copy(ln["xbd"][0:64, 0:64],
                                      ln["xstk"][0:64, :])
                nc.gpsimd.tensor_copy(ln["xbd"][64:128, 64:128],
                                      ln["xstk"][64:128, :])
                if L < 2:
                    # R^T = X C^T (X symmetric) -> bdl blocks
                    rts = ns_ps.tile([128, 64], F32, tag=f"nsps{L}", bufs=2)
                    nc.tensor.matmul(rts[:], ln["xbd"][:], ln["ctstk"][:])
                    nc.vector.tensor_copy(bdl[L][0:64, 0:64], rts[0:64, :])
                    nc.scalar.copy(bdl[L][64:128, 64:128], rts[64:128, :])
                else:
                    # R = C X = (ctbd)^T X -> bdr blocks (replicated)
                    ctbd = ns_sb.tile([128, 128], F32, tag=f"ctbd{L}")
                    nc.gpsimd.tensor_copy(ctbd[:], zeros[:])
                    nc.gpsimd.tensor_copy(ctbd[0:64, 0:64],
                                          ln["ctstk"][0:64, :])
                    nc.gpsimd.tensor_copy(ctbd[64:128, 64:128],
                                          ln["ctstk"][64:128, :])
                    rs = ns_ps.tile([128, 64], F32, tag=f"nsps{L}", bufs=2)
                    nc.tensor.matmul(rs[:], ctbd[:], ln["xstk"][:])
                    ca, cb = 2 * (L - 2), 2 * (L - 2) + 1
                    nc.vector.tensor_copy(bdr[ca][0:64, 0:64], rs[0:64, :])
                    nc.scalar.copy(bdr[ca][64:128, 64:128], rs[0:64, :])
                    nc.vector.tensor_copy(bdr[cb][0:64, 0:64], rs[64:128, :])
                    nc.scalar.copy(bdr[cb][64:128, 64:128], rs[64:128, :])

        # ---------------- Phase 2: main loop (all bf16) ----------------
        io_pool = ctx.enter_context(tc.tile_pool(name="io", bufs=in_bufs))
        out_pool = ctx.enter_context(tc.tile_pool(name="outp", bufs=out_bufs))
        mid_pool = ctx.enter_context(tc.tile_pool(name="mid", bufs=mid_bufs))
        ps_pool = ctx.enter_context(
            tc.tile_pool(name="mainps", bufs=1, space="PSUM"))

        PSTRIDE = B * W             # partition stride in wkv elements
        for g in range(NG):
            # ---- input DMA: per pair, [128, 1024] (2KB lines) ----
            xin = []
            for P in range(2):
                t = io_pool.tile([128, 1024], BF16, tag=f"xin{P}")
                off = (2 * P) * H * PSTRIDE + g * GB * W
                nc.sync.dma_start(
                    t[:], bass.AP(wkv, off, [[PSTRIDE, 128], [1, 1024]]))
                xin.append(t)

            # ---- MM1: Y = R1 X, pair-stacked, K=128 ----
            ysb = [mid_pool.tile([128, 1024], BF16, tag=f"ysb{P}",
                                 name=f"ysb{P}_{g}") for P in range(2)]
            cp = 0
            for P in range(2):
                for h in range(2):
                    yps = ps_pool.tile([128, 512], F32, tag="mm1", bufs=2)
                    nc.tensor.matmul(yps[:], bdl[P][:],
                                     xin[P][:, 512 * h:512 * h + 512])
                    eng = (nc.scalar, nc.vector)[cp % 2]
                    cp += 1
                    _ecopy(eng, ysb[P][:, 512 * h:512 * h + 512], yps[:])

            # ---- T1: bf16 PE transposes of [128,128] blocks ----
            ytsb = [mid_pool.tile([128, 1024], BF16, tag=f"ytsb{P}",
                                  name=f"ytsb{P}_{g}") for P in range(2)]
            for P in range(2):
                for hh in range(2):
                    tps = ps_pool.tile([128, 512], BF16, tag="t1", bufs=2)
                    for qq in range(4):
                        q = 4 * hh + qq
                        nc.tensor.transpose(
                            tps[:, 128 * qq:128 * qq + 128],
                            ysb[P][:, 128 * q:128 * q + 128], identb[:])
                    nc.vector.tensor_copy(
                        ytsb[P][:, 512 * hh:512 * hh + 512], tps[:])

            # ---- MM2: Z^T = R2^T Y^T, batch-pair-stacked, K=128 ----
            zsb = out_pool.tile([128, 2048], BF16, tag="zsb")
            cp = 0
            for P in range(2):
                for cc in range(2):
                    c = 2 * P + cc
                    zps = ps_pool.tile([128, 512], F32, tag="mm2", bufs=2)
                    base = ytsb[P][:, 64 * cc:64 * cc + 64]
                    rhs = bass.AP(base.tensor, base.offset,
                                  [list(base.ap[0]), [128, 8], [1, 64]])
                    nc.tensor.matmul(zps[:], bdr[c][:], rhs)
                    eng = (nc.scalar, nc.vector, nc.scalar, nc.scalar)[cp]
                    cp += 1
                    _ecopy(eng, zsb[:, 512 * c:512 * c + 512], zps[:])

            nc.sync.dma_start(
                bass.AP(out, g * 128 * 2048, [[2048, 128], [1, 2048]]),
                zsb[:])

    return nc


def _unscramble(scr):
    """scr [NG, 128, 2048] bf16 -> [512, 4, 64, 64] f32.
    scr[g, 64b + j', 512c + 64q + i] = Z[16g + 2q + b, c][i, j']."""
    a = np.asarray(scr).astype(np.float32)
    a = a.reshape(NG, 2, 64, HPC, 8, 64)        # g, b, j', c, q, i
    a = a.transpose(0, 4, 1, 3, 5, 2)           # g, q, b, c, i, j'
    return np.ascontiguousarray(a.reshape(B, HPC, H, W))


def _make_in_maps(wkv, p_left, p_right):
    in_maps = []
    for k in range(N_CORES):
        sl = slice(HPC * k, HPC * k + HPC)
        # [b, c, i, j] -> [c, i, b, j], cast bf16
        wt = wkv[:, sl].transpose(1, 2, 0, 3)
        in_maps.append({
            "wkv": wt.astype(BF_NP),
            "p_left": np.ascontiguousarray(p_left[sl]),
            "p_right": np.ascontiguousarray(p_right[sl]),
        })
    return in_maps


_CACHED = {}


def _get_program():
    if "nc" not in _CACHED:
        _CACHED["nc"] = build()
    return _CACHED["nc"]


def kernel(wkv, p_left, p_right):
    wkv = np.ascontiguousarray(wkv, dtype=np.float32)
    p_left = np.ascontiguousarray(p_left, dtype=np.float32)
    p_right = np.ascontiguousarray(p_right, dtype=np.float32)
    assert wkv.shape == (B, 32, H, W), wkv.shape

    nc = _get_program()
    in_maps = _make_in_maps(wkv, p_left, p_right)
    res = run_bass_kernel_spmd(nc, in_maps, list(range(N_CORES)))
    return np.concatenate(
        [_unscramble(np.asarray(res.results[k]["out_scr"]))
         for k in range(N_CORES)], axis=1)


# revision 9
# speedup vs baseline: 2.1225x; 1.1981x over previous
"""BilateralRotation Trainium2 kernel: out[b,c] = R1[c] @ wkv[b,c] @ R2[c],
R = Cayley(p) = (I - A)(I + A)^-1, A = 0.5(p - p^T).

Sharding: 8 NeuronCores, head-parallel - core k owns heads [4k, 4k+4) for all
512 batches. bf16 end-to-end on the data path (rel-err budget 2e-2; measured
~4.5e-3): the host converts the wkv shard to bf16 in a [c, i, b, j] layout so
every DMA line is a 2KB contiguous run, and the device writes bf16 results
that the host casts back to fp32.

Device program per core:
  Phase 1 - Cayley via Newton-Schulz in fp32, 4 pair-packed lanes
    (two 64x64 matrices stacked on 128 partitions; block-diagonal stationaries
    so each NS step is ONE matmul per product):
      B = A^T A, M = I + B, X' = 2X - X(MX), 12 iters, X0 = I/300.
      X is symmetric (polynomial in M), so no X^T tracking is needed.
      R1 path stores R^T = X C^T, R2 path stores R = C X (C^T = I + 2A - B).
  Phase 2 - main loop over 32 groups of 16 batches, all-bf16:
    - MM1: K=128 block-diag stationary blockdiag(R1_c0^T, R1_c1^T) per head
      pair; moving = xin [128, 512] -> Y pair-stacked in PSUM (1 col/cycle,
      full PE).
    - T1: PE transposes of bf16 [128,128] blocks (1 cyc/row vs 2 for fp32).
    - MM2: stationary blockdiag(R2_c, R2_c) (batch-pair K packing), moving
      gathers head c' columns via a strided AP, N=512.
    - PSUM->SBUF copies split across DVE / Act / Pool engines.
    - zsb [128, 2048] bf16 dumped contiguously; host inverts the fixed index
      permutation while unsharding.
"""

import sys
import types
from contextlib import ExitStack

import numpy as np
import ml_dtypes

# ---------------------------------------------------------------------------
# TileContext patch: this walrus build accepts only ONE sync-wait per
# instruction; hoist extra waits onto nops inserted before the instruction.
# ---------------------------------------------------------------------------
import concourse.bass as bass
import concourse.tile as tile
from concourse.vector_clock import ScopedClock
from concourse import masks, mybir
from concourse.bass_utils import run_bass_kernel_spmd

WAIT_LIMIT = 1


def _hoist_extra_waits(nc, inst, hint):
    nops = []
    si = inst.sync_info
    if si is not None and len(si.on_wait) > WAIT_LIMIT:
        extras = si.on_wait[:-WAIT_LIMIT]
        del si.on_wait[:-WAIT_LIMIT]
        for w in extras:
            nop = nc.engines[inst.engine].nop(nofuse=True, hint=hint)
            nsi = nop.ins.sync_info
            if nsi is None:
                nop.ins.sync_info = mybir.SyncInfo(on_wait=[w], on_update=[])
            else:
                nsi.on_wait.append(w)
            nops.append(nop.ins)
    return nops


def _split_waits(nc):
    cur_list = nc.cur_bb.bb.instructions
    for f in nc.m.functions:
        for bb in f.blocks:
            orig = list(bb.instructions)
            if not any(i.sync_info and len(i.sync_info.on_wait) > WAIT_LIMIT
                       for i in orig):
                continue
            new_list = []
            for inst in orig:
                nops = _hoist_extra_waits(nc, inst, "split_wait")
                for nop in nops:
                    if cur_list and cur_list[-1] is nop:
                        cur_list.pop()
                    else:
                        cur_list.remove(nop)
                new_list.extend(nops)
                new_list.append(inst)
            bb.instructions[:] = new_list


def _drain_and_barrier(self, tick_clock, wait_clock):
    nc = self.nc
    _split_waits(nc)
    drain_inst = nc.sync.drain()
    wait_clock.add_sem_waits(drain_inst.ins,
                             ScopedClock({None: tick_clock.global_clock}))
    nops = _hoist_extra_waits(nc, drain_inst.ins, "drain_split_wait")
    if nops:
        insts = nc.cur_bb.bb.instructions
        di = insts.index(drain_inst.ins)
        insts.append(insts.pop(di))
    nc.all_engine_barrier()
    assert self.sems is not None
    popped = nc._tile_sem_poison_stack.pop()
    assert popped is self._sem_poison
    nc.clear_and_free_semaphores(list(self.sems.allocated().values()))
    nc.all_engine_barrier()


tile.TileContext._drain_and_barrier = _drain_and_barrier

# ---------------------------------------------------------------------------
# Program builder
# ---------------------------------------------------------------------------
dt = mybir.dt
F32 = dt.float32
BF16 = dt.bfloat16
BF_NP = np.dtype(ml_dtypes.bfloat16)

HPC = 4                     # heads per core
B = 512
H = W = 64
NG = 32                     # batch groups of 16
GB = 16                     # batches per group
N_CORES = 8
NS_ITERS = 12
C0 = 1.0 / 300.0



def _ecopy(eng, dst, src):
    if hasattr(eng, "tensor_copy"):
        eng.tensor_copy(dst, src)
    else:
        eng.copy(dst, src)


def build(in_bufs=12, out_bufs=3, mid_bufs=2):
    nc = bass.Bass("TRN2", target_bir_lowering=False, debug=False,
                   num_devices=N_CORES)
    # [c, i, b, j] bf16 layout: partition lines are 2KB contiguous runs.
    wkv = nc.dram_tensor("wkv", [HPC, H, B, W], BF16, kind="ExternalInput")
    p_left = nc.dram_tensor("p_left", [HPC, H, H], F32, kind="ExternalInput")
    p_right = nc.dram_tensor("p_right", [HPC, W, W], F32,
                             kind="ExternalInput")
    out = nc.dram_tensor("out_scr", [NG, 128, 2048], BF16,
                         kind="ExternalOutput")

    with tile.TileContext(nc) as tc, ExitStack() as ctx:
        const_pool = ctx.enter_context(tc.tile_pool(name="const", bufs=1))
        bd_pool = ctx.enter_context(tc.tile_pool(name="bd", bufs=1))

        ident = const_pool.tile([128, 128], F32, tag="ident")
        masks.make_identity(nc, ident[:])
        i64 = ident[0:64, 0:64]
        identb = const_pool.tile([128, 128], BF16, tag="identb")
        nc.vector.tensor_copy(identb[:], ident[:])
        # [I64; I64] stacked (both 64x64 blocks of the first 64 cols)
        istk = const_pool.tile([128, 64], F32, tag="istk")
        nc.vector.tensor_copy(istk[0:64, :], i64)
        nc.scalar.copy(istk[64:128, :], i64)

        # main-loop stationaries (bf16): bdl[P] = blockdiag(R1_{2P}^T,
        # R1_{2P+1}^T); bdr[c] = blockdiag(R2_c, R2_c)
        bdl = []
        for P in range(2):
            t = bd_pool.tile([128, 128], BF16, tag=f"bdl{P}")
            nc.gpsimd.memset(t[:], 0.0)
            bdl.append(t)
        bdr = []
        for c in range(HPC):
            t = bd_pool.tile([128, 128], BF16, tag=f"bdr{c}")
            nc.gpsimd.memset(t[:], 0.0)
            bdr.append(t)

        # ---------------- Phase 1: Newton-Schulz Cayley (pair lanes) -------
        # lane 0,1: p_left pairs (R1); lane 2,3: p_right pairs (R2)
        # (Activation engine has no tensor-tensor ops; arithmetic alternates
        # DVE / Pool per lane, pure copies go to Activation.)
        ENGS = [nc.vector, nc.gpsimd]

        with ExitStack() as nsctx:
            ns_sb = nsctx.enter_context(tc.tile_pool(name="ns_sb", bufs=2))
            ns_keep = nsctx.enter_context(tc.tile_pool(name="ns_keep",
                                                       bufs=2))
            ns_ps = nsctx.enter_context(
                tc.tile_pool(name="ns_ps", bufs=1, space="PSUM"))

            lanes = []
            for L in range(4):
                src_t = p_left if L < 2 else p_right
                ca, cb = 2 * (L % 2), 2 * (L % 2) + 1

                # side-by-side [64, 128] for one transpose; stacked [128, 64]
                pside = ns_sb.tile([64, 128], F32, tag=f"pside{L}")
                nc.sync.dma_start(pside[:, 0:64], src_t.ap()[ca])
                nc.sync.dma_start(pside[:, 64:128], src_t.ap()[cb])
                pstk = ns_sb.tile([128, 64], F32, tag=f"pstk{L}")
                nc.sync.dma_start(pstk[0:64, :], src_t.ap()[ca])
                nc.sync.dma_start(pstk[64:128, :], src_t.ap()[cb])

                ptp = ns_ps.tile([128, 64], F32, tag=f"nsps{L}", bufs=2)
                nc.tensor.transpose(ptp[:], pside[:], i64)

                astk = ns_keep.tile([128, 64], F32, tag=f"astk{L}")
                nc.vector.tensor_sub(astk[:], pstk[:], ptp[:])
                nc.vector.tensor_scalar_mul(astk[:], astk[:], 0.5)

                # B = A^T A per half via quadrant matmuls (tile_position
                # inferred from the partition offsets)
                bps = ns_ps.tile([128, 64], F32, tag=f"nsps{L}", bufs=2)
                nc.tensor.matmul(bps[0:64, :], astk[0:64, :], astk[0:64, :])
                nc.tensor.matmul(bps[64:128, :], astk[64:128, :],
                                 astk[64:128, :])

                mstk = ns_keep.tile([128, 64], F32, tag=f"mstk{L}")
                nc.vector.tensor_add(mstk[:], bps[:], istk[:])  # M = I + B

                ctstk = ns_keep.tile([128, 64], F32, tag=f"ct{L}")
                nc.vector.scalar_tensor_tensor(                # 2A - B
                    ctstk[:], astk[:], 2.0, bps[:],
                    op0=mybir.AluOpType.mult,
                    op1=mybir.AluOpType.subtract)
                nc.vector.tensor_add(ctstk[:], ctstk[:], istk[:])   # C^T

                xstk = ns_keep.tile([128, 64], F32, tag=f"x{L}")
                nc.vector.tensor_scalar_mul(xstk[:], istk[:], C0)

                lanes.append(dict(xstk=xstk, mstk=mstk, ctstk=ctstk))

            def qmm(out, s, m):
                nc.tensor.matmul(out[0:64, :], s[0:64, :], m[0:64, :])
                nc.tensor.matmul(out[64:128, :], s[64:128, :], m[64:128, :])

            for k in range(NS_ITERS):
                for L, ln in enumerate(lanes):
                    pps = ns_ps.tile([128, 64], F32, tag=f"nsps{L}", bufs=2)
                    qmm(pps, ln["mstk"], ln["xstk"])            # M X (M sym)
                    psb = ns_sb.tile([128, 64], F32, tag=f"psb{L}")
                    nc.scalar.copy(psb[:], pps[:])
                    wps = ns_ps.tile([128, 64], F32, tag=f"nsps{L}", bufs=2)
                    qmm(wps, ln["xstk"], psb)                   # X (MX) (X sym)
                    xnew = ns_keep.tile([128, 64], F32, tag=f"x{L}")
                    nc.vector.scalar_tensor_tensor(            # 2X - X(MX)
                        xnew[:], ln["xstk"][:], 2.0, wps[:],
                        op0=mybir.AluOpType.mult,
                        op1=mybir.AluOpType.subtract)
                    ln["xstk"] = xnew

            for L, ln in enumerate(lanes):
                rs = ns_ps.tile([128, 64], F32, tag=f"nsps{L}", bufs=2)
                if L < 2:
                    # R^T = X C^T (X symmetric) -> bdl blocks
                    qmm(rs, ln["xstk"], ln["ctstk"])
                    nc.vector.tensor_copy(bdl[L][0:64, 0:64], rs[0:64, :])
                    nc.scalar.copy(bdl[L][64:128, 64:128], rs[64:128, :])
                else:
                    # R = C X = (C^T)^T X -> bdr blocks (replicated)
                    qmm(rs, ln["ctstk"], ln["xstk"])
                    ca, cb = 2 * (L - 2), 2 * (L - 2) + 1
                    nc.vector.tensor_copy(bdr[ca][0:64, 0:64], rs[0:64, :])
                    nc.scalar.copy(bdr[ca][64:128, 64:128], rs[0:64, :])
                    nc.vector.tensor_copy(bdr[cb][0:64, 0:64], rs[64:128, :])
                    nc.scalar.copy(bdr[cb][64:128, 64:128], rs[64:128, :])

        # ---------------- Phase 2: main loop (all bf16) ----------------
        io_pool = ctx.enter_context(tc.tile_pool(name="io", bufs=in_bufs))
        out_pool = ctx.enter_context(tc.tile_pool(name="outp", bufs=out_bufs))
        mid_pool = ctx.enter_context(tc.tile_pool(name="mid", bufs=mid_bufs))
        ps_pool = ctx.enter_context(
            tc.tile_pool(name="mainps", bufs=1, space="PSUM"))

        PSTRIDE = B * W             # partition stride in wkv elements
        for g in range(NG):
            # ---- input DMA: per pair, [128, 1024] (2KB lines) ----
            xin = []
            for P in range(2):
                t = io_pool.tile([128, 1024], BF16, tag=f"xin{P}")
                off = (2 * P) * H * PSTRIDE + g * GB * W
                nc.sync.dma_start(
                    t[:], bass.AP(wkv, off, [[PSTRIDE, 128], [1, 1024]]))
                xin.append(t)

            # ---- MM1: Y = R1 X, pair-stacked, K=128 ----
            ysb = [mid_pool.tile([128, 1024], BF16, tag=f"ysb{P}",
                                 name=f"ysb{P}_{g}") for P in range(2)]
            mm1_eng = ([nc.scalar, nc.scalar, nc.vector, nc.scalar] if g % 2
                       else [nc.scalar, nc.vector, nc.scalar, nc.scalar])
            cp = 0
            for P in range(2):
                for h in range(2):
                    yps = ps_pool.tile([128, 512], F32, tag="mm1", bufs=3)
                    nc.tensor.matmul(yps[:], bdl[P][:],
                                     xin[P][:, 512 * h:512 * h + 512])
                    _ecopy(mm1_eng[cp], ysb[P][:, 512 * h:512 * h + 512],
                           yps[:])
                    cp += 1

            # ---- T1: bf16 PE transposes of [128,128] blocks ----
            ytsb = [mid_pool.tile([128, 1024], BF16, tag=f"ytsb{P}",
                                  name=f"ytsb{P}_{g}") for P in range(2)]
            for P in range(2):
                tps = ps_pool.tile([128, 1024], BF16, tag="t1", bufs=2)
                for q in range(8):
                    nc.tensor.transpose(
                        tps[:, 128 * q:128 * q + 128],
                        ysb[P][:, 128 * q:128 * q + 128], identb[:])
                nc.vector.tensor_copy(ytsb[P][:], tps[:])

            # ---- MM2: Z^T = R2^T Y^T, batch-pair-stacked, K=128 ----
            zsb = out_pool.tile([128, 2048], BF16, tag="zsb")
            mm2_eng = ([nc.vector, nc.scalar, nc.scalar, nc.vector] if g % 2
                       else [nc.vector, nc.scalar, nc.scalar, nc.scalar])
            cp = 0
            for P in range(2):
                for cc in range(2):
                    c = 2 * P + cc
                    zps = ps_pool.tile([128, 512], F32, tag="mm2", bufs=3)
                    base = ytsb[P][:, 64 * cc:64 * cc + 64]
                    rhs = bass.AP(base.tensor, base.offset,
                                  [list(base.ap[0]), [128, 8], [1, 64]])
                    nc.tensor.matmul(zps[:], bdr[c][:], rhs)
                    _ecopy(mm2_eng[cp], zsb[:, 512 * c:512 * c + 512], zps[:])
                    cp += 1

            nc.sync.dma_start(
                bass.AP(out, g * 128 * 2048, [[2048, 128], [1, 2048]]),
                zsb[:])

    return nc


def _unscramble(scr):
    """scr [NG, 128, 2048] bf16 -> [512, 4, 64, 64] f32.
    scr[g, 64b + j', 512c + 64q + i] = Z[16g + 2q + b, c][i, j']."""
    a = np.asarray(scr).astype(np.float32)
    a = a.reshape(NG, 2, 64, HPC, 8, 64)        # g, b, j', c, q, i
    a = a.transpose(0, 4, 1, 3, 5, 2)           # g, q, b, c, i, j'
    return np.ascontiguousarray(a.reshape(B, HPC, H, W))


def _make_in_maps(wkv, p_left, p_right):
    in_maps = []
    for k in range(N_CORES):
        sl = slice(HPC * k, HPC * k + HPC)
        # [b, c, i, j] -> [c, i, b, j], cast bf16
        wt = wkv[:, sl].transpose(1, 2, 0, 3)
        in_maps.append({
            "wkv": wt.astype(BF_NP),
            "p_left": np.ascontiguousarray(p_left[sl]),
            "p_right": np.ascontiguousarray(p_right[sl]),
        })
    return in_maps


_CACHED = {}


def _get_program():
    if "nc" not in _CACHED:
        _CACHED["nc"] = build()
    return _CACHED["nc"]


def kernel(wkv, p_left, p_right):
    wkv = np.ascontiguousarray(wkv, dtype=np.float32)
    p_left = np.ascontiguousarray(p_left, dtype=np.float32)
    p_right = np.ascontiguousarray(p_right, dtype=np.float32)
    assert wkv.shape == (B, 32, H, W), wkv.shape

    nc = _get_program()
    in_maps = _make_in_maps(wkv, p_left, p_right)
    res = run_bass_kernel_spmd(nc, in_maps, list(range(N_CORES)))
    return np.concatenate(
        [_unscramble(np.asarray(res.results[k]["out_scr"]))
         for k in range(N_CORES)], axis=1)
